# revision 47
# baseline (speedup 1.0000x reference)
"""Trainium2 Bass kernel for the Performer-style random-feature map:

    out[n, s] = exp(-||x_n||^2 / 2) * S^{-1/2} * exp((x @ W.T)[n, s] + b[s])
              = exp((x @ W.T)[n, s] - 0.5*||x_n||^2 - 0.5*ln(S)) * exp(b[s])

Sharding: data-parallel over the N (row) axis across 8 NeuronCores; W and b
replicated.  Each core computes a [2048, 2048] output block.  Pure SPMD, no
collectives.

Per-core structure (sizes hardcoded for N=16384, D=1024, S=2048):
  - x^T and W^T live in SBUF as fp8e4 (W pre-scaled by 16 on the host to
    stay out of the fp8 subnormal range); matmuls run in
    MatmulPerfMode.DoubleRow: two 128-deep k-subtiles per instruction,
    2x bf16 throughput (a [128,512] K=256 tile issues every ~216ns) --
    but ONLY when the rhs k-pair slice is contiguous in SBUF, hence the
    w_sb layout [P, colgrp, KT, 512].
  - the per-row bias -0.5*||x_n||^2 - 0.5*ln(S) rides in as a tiny
    host-packed [128, NB] f32 tensor, so no second copy of x is loaded.
  - all bulk DMA rides the sync ring, demand-ordered (first blocks' x,
    W phase-0 columns, rest of x, W phase-1) in ~128-256KB chunks;
    subtile dependency tracking lets each matmul wait only on the chunk
    it reads.  The scalar (ACT) engine issues nothing but two tiny
    loads before its exp stream -- a bulk dma_start there would stall
    PSUM recycling behind DMA completion-lane waits.  Half the out
    DMAs ride scalar anyway, issued two tiles late so the data is
    always ready.  A few dummy warmup matmuls absorb the PE pstate
    ramp while the first operands arrive (any PE idle gap resets the
    ramp and costs double).
  - column-phase sweep (phase h = out[:, h*1024:(h+1)*1024] for all 16
    row blocks) so the stream is gated on 1MB of W, not 2MB; per
    [128, 1024] PSUM tile: 8 DoubleRow matmuls -> one ACT
    exp(psum/16 + bias_n) -> bf16 tmp -> DVE multiply by exp(b) ->
    bf16 DMA out (host upcasts to f32).  The final tile drains at
    256-col granularity across both rings to shorten the tail.
"""

import sys
from contextlib import ExitStack

if "/opt/trn_rl_repo" not in sys.path:
    sys.path.insert(0, "/opt/trn_rl_repo")

import numpy as np

import concourse.bacc as bacc
import concourse.bass as bass
import concourse.tile as tile
from concourse import mybir

P = 128          # SBUF partitions
N_FULL = 16384   # total rows
D_FULL = 1024    # contraction dim
S_FULL = 2048    # output features
N_CORES = 8
NC_FULL = N_FULL // N_CORES  # rows per core

F32 = mybir.dt.float32
BF16 = mybir.dt.bfloat16
FP8 = mybir.dt.float8e4

W_SCALE = 16.0   # host multiplies W by this before fp8 cast


def build_nc(NCc=NC_FULL, D=D_FULL, S=S_FULL, psum_w=1024, warmup=8,
             mul_wide=True):
    """Build the single-core Bass program (same program runs SPMD on 8 cores)."""
    nc = bacc.Bacc("TRN2", target_bir_lowering=False, debug=False)

    xT = nc.dram_tensor("xT", [D, NCc], FP8, kind="ExternalInput").ap()
    w = nc.dram_tensor("w", [D, S], FP8, kind="ExternalInput").ap()
    bv = nc.dram_tensor("bias", [S], F32, kind="ExternalInput").ap()
    # host-packed [-0.5*||x_n||^2 - 0.5*ln(S)] as [P, NB]
    rb = nc.dram_tensor("rowbias", [P, NCc // P], F32,
                        kind="ExternalInput").ap()
    out = nc.dram_tensor("out", [NCc, S], BF16, kind="ExternalOutput").ap()

    KT = D // P            # k subtiles (contraction)
    KP = KT // 2           # k pairs (DoubleRow consumes 2 subtiles)
    NB = NCc // P          # 128-row output blocks
    NS = 512               # matmul moving free width (f32 psum half-bank pair)
    SW = psum_w            # psum tile width (2 banks)
    SH = S // SW           # psum tiles per row block
    CH = SW // NS          # matmul column groups per psum tile
    DR = mybir.MatmulPerfMode.DoubleRow

    with tile.TileContext(nc) as tc, ExitStack() as ctx:
        singles = ctx.enter_context(tc.tile_pool(name="singles", bufs=1))
        # w layout keeps the matmul rhs slice [2, 512] contiguous (the moving
        # feed needs adjacent k-pairs to double-pump); x keeps 2KB DMA runs:
        #   w_sb[p, c, k, j] = W[k*128+p, c*512+j]
        #   x_sb[p, k, n]    = x[n, k*128+p]
        w_sb = singles.tile([P, S // NS, KT, NS], FP8)
        x_sb = singles.tile([P, KT, NCc], FP8)
        b_bc = singles.tile([P, S], F32)
        eb = singles.tile([P, S], BF16)
        rb_sb = singles.tile([P, NB], F32)

        # --- DMA issue ---
        # The scalar (ACT) engine issues ONLY the two tiny loads, then eb,
        # then the per-tile exps: any bulk dma_start on its in-order stream
        # would block the first exp behind DMA completion-lane waits and
        # stall PSUM recycling.  The sync ring carries every bulk transfer.
        bv_bcast = bass.AP(tensor=bv.tensor, offset=bv.offset,
                           ap=[[0, P]] + list(bv.ap))
        nc.scalar.dma_start(b_bc, bv_bcast)
        nc.scalar.dma_start(rb_sb, rb)
        nc.scalar.activation(eb, b_bc, func=mybir.ActivationFunctionType.Exp)
        # Need order on the sync ring: the first row-blocks' x, W phase-0
        # columns (they gate the whole stream), the rest of x, then W
        # phase-1 (not needed until the second half of the compute).
        cph = (S // NS) // SH
        def w_chunk(ph, k):
            nc.sync.dma_start(
                w_sb[:, ph * cph:(ph + 1) * cph, k, :],
                w[k * P:(k + 1) * P, ph * cph * NS:(ph + 1) * cph * NS]
                .rearrange("p (c j) -> p c j", j=NS))
        XC = 2 * P                  # x columns on the critical path
        nc.sync.dma_start(
            x_sb[:, 0:KT // 2, 0:XC],
            xT[0:KT // 2 * P, 0:XC].rearrange("(k p) n -> p k n", p=P))
        w_chunk(0, 0)
        w_chunk(0, 1)
        nc.sync.dma_start(
            x_sb[:, KT // 2:KT, 0:XC],
            xT[KT // 2 * P:KT * P, 0:XC].rearrange("(k p) n -> p k n", p=P))
        for k in range(2, KT):
            w_chunk(0, k)
        # Remaining x in four all-k n-quarter sweeps: every block needs all
        # k of its n-range, so merging k into one DMA loses no dependency
        # granularity and quarters the enqueue count (outs start earlier).
        xm = (NCc - XC) // 4
        for q in range(4):
            lo = XC + q * xm
            nc.sync.dma_start(
                x_sb[:, :, lo:lo + xm],
                xT[:, lo:lo + xm].rearrange("(k p) n -> p k n", p=P))
        for k in range(KT):
            w_chunk(1, k)

        psum_pool = ctx.enter_context(
            tc.tile_pool(name="psum", bufs=8 * 512 // SW, space="PSUM"))
        tmp_pool = ctx.enter_context(tc.tile_pool(name="tmp", bufs=7))
        out_pool = ctx.enter_context(tc.tile_pool(name="osb", bufs=8))

        if warmup:
            # keep the PE busy (and the pstate ramping) while the first
            # operand chunks stream in; results are discarded
            dummy_x = singles.tile([P, 2, P], FP8)
            dummy_w = singles.tile([P, 2, NS], FP8)
            nc.vector.memset(dummy_x, 0.0)
            nc.vector.memset(dummy_w, 0.0)
            for i in range(warmup):
                wps = psum_pool.tile([P, SW], F32, tag="ps", name=f"warm{i}")
                nc.tensor.matmul(wps[:, 0:NS], lhsT=dummy_x, rhs=dummy_w,
                                 start=True, stop=True, perf_mode=DR)

        # Column-phase sweep: phase h computes out[:, h*SW:(h+1)*SW] for all
        # row blocks, so the stream is gated only on W's phase-0 columns.
        # Half the out-DMAs ride the scalar ring, issued two tiles late so
        # the exp stream never blocks on a DMA completion-lane wait.
        tiles = [(h, nb) for h in range(SH) for nb in range(NB)]
        T = len(tiles)
        pend = {}
        for t, (h, nb) in enumerate(tiles):
            last = t == T - 1
            ps = psum_pool.tile([P, SW], F32, tag="ps", name=f"ps{h}_{nb}")
            order = ([(kp, c) for c in range(CH) for kp in range(KP)]
                     if last else
                     [(kp, c) for kp in range(KP) for c in range(CH)])
            for kp, c in order:
                nc.tensor.matmul(
                    ps[:, c * NS:(c + 1) * NS],
                    lhsT=x_sb[:, 2 * kp:2 * kp + 2, nb * P:(nb + 1) * P],
                    rhs=w_sb[:, h * CH + c, 2 * kp:2 * kp + 2, :],
                    start=(kp == 0),
                    stop=(kp == KP - 1),
                    perf_mode=DR,
                )
            sl = slice(h * SW, (h + 1) * SW)
            if not last:
                tmp = tmp_pool.tile([P, SW], BF16)
                nc.scalar.activation(
                    tmp, ps,
                    func=mybir.ActivationFunctionType.Exp,
                    bias=rb_sb[:, nb:nb + 1],
                    scale=1.0 / W_SCALE,
                )
                if t - 2 in pend:
                    nc.scalar.dma_start(*pend.pop(t - 2))
                o_sb = out_pool.tile([P, SW], BF16)
                nc.vector.tensor_mul(o_sb, tmp, eb[:, sl])
                dst = out[nb * P:(nb + 1) * P, sl]
                if t % 2 == 0:
                    nc.sync.dma_start(dst, o_sb)
                else:
                    pend[t] = (dst, o_sb)
            else:
                # flush any still-pending delayed out first: its data has
                # been ready for two tiles, and enqueueing it after the
                # drain pieces would put a full 256KB transfer dead last
                for tp in sorted(pend):
                    nc.scalar.dma_start(*pend.pop(tp))
                # fine-grained drain: the last tile's exp/mul/DMA run at 512
                # (and 256) granularity over both rings so the final
                # write-back latency chain is as short as possible
                for cc in range(CH):
                    csl = slice(h * SW + cc * NS, h * SW + (cc + 1) * NS)
                    tmp = tmp_pool.tile([P, NS], BF16, tag="tmpl",
                                        name=f"tl{cc}")
                    nc.scalar.activation(
                        tmp, ps[:, cc * NS:(cc + 1) * NS],
                        func=mybir.ActivationFunctionType.Exp,
                        bias=rb_sb[:, nb:nb + 1],
                        scale=1.0 / W_SCALE,
                    )
                    o_sb = out_pool.tile([P, NS], BF16, tag="osl",
                                         name=f"ol{cc}")
                    nc.vector.tensor_mul(o_sb, tmp, eb[:, csl])
                    for sub in range(2):
                        lo = csl.start + sub * NS // 2
                        eng = nc.sync if sub == 0 else nc.scalar
                        eng.dma_start(
                            out[nb * P:(nb + 1) * P, lo:lo + NS // 2],
                            o_sb[:, sub * NS // 2:(sub + 1) * NS // 2])
        for t in sorted(pend):
            nc.scalar.dma_start(*pend.pop(t))

    nc.compile()
    return nc


_NC_CACHE = {}


def _get_nc(**kwargs):
    key = tuple(sorted(kwargs.items()))
    if key not in _NC_CACHE:
        _NC_CACHE[key] = build_nc(**kwargs)
    return _NC_CACHE[key]


def make_in_maps(x, W, b):
    import ml_dtypes
    fp8 = ml_dtypes.float8_e4m3fn
    NB = NC_FULL // P
    wT = np.ascontiguousarray((W.T * W_SCALE).astype(fp8))
    b = np.ascontiguousarray(b.astype(np.float32))
    in_maps = []
    for i in range(N_CORES):
        xs = x[i * NC_FULL:(i + 1) * NC_FULL].astype(np.float32)
        rowbias = (-0.5 * (xs * xs).sum(axis=1)
                   - 0.5 * np.log(S_FULL)).astype(np.float32)
        in_maps.append({
            "xT": np.ascontiguousarray(xs.T.astype(fp8)),
            "w": wT,
            "bias": b,
            "rowbias": np.ascontiguousarray(rowbias.reshape(NB, P).T),
        })
    return in_maps


def run_hw(x, W, b, trace=False, **build_kwargs):
    """Run on 8 NeuronCores; returns (out [N, S] f32, BassKernelResults)."""
    from concourse.bass_utils import run_bass_kernel_spmd
    from concourse.bass_interp import get_hw_module

    nc = _get_nc(**build_kwargs)
    in_maps = make_in_maps(x, W, b)
    old_m = nc.m
    nc.m = get_hw_module(nc.m)
    try:
        res = run_bass_kernel_spmd(
            nc, in_maps, core_ids=list(range(N_CORES)), trace=trace)
    finally:
        nc.m = old_m
    out = np.concatenate(
        [res.results[i]["out"].astype(np.float32) for i in range(N_CORES)],
        axis=0)
    return out, res


def kernel(x, W, b):
    out, _ = run_hw(x, W, b, trace=False)
    return out
